# revision 33
# baseline (speedup 1.0000x reference)
"""Detection-loss kernel for Trainium2 (8 NeuronCores, data-parallel over B).

Reference: scatter 64 targets/image into a [B,C,H,W] map + mask, then
masked SmoothL1(preds, map).sum() / num_objects.  The mask is nonzero at
<= B*T cells, so the loss only depends on preds at those cells: each core
*gathers* preds at its 4 images' (gy,gx) cells instead of streaming 18MB.

Device program (raw Bass, hand-placed semaphores, no TileContext, no
nc.Block — engine streams are emitted straight into the main bb so the
redundant block-exit all-engine barrier disappears; the runtime's own
end-of-kernel handshake synchronizes engines):
  - two indirect DMAs (128 descriptors each, 32B slots, channels-last
    relayout so one descriptor moves all 7 channels).  HW indirect DMA
    emits exactly ONE descriptor per partition (extra offset columns are
    silently ignored - CoreSim models them, HW does not), so 256 targets
    need two calls.  They run on SEPARATE SWDGE queues (num_swdge_queues=2,
    second call hand-lowered onto qPoolDynamic1): on a shared queue both
    completion sems deliver only ~2us after the LAST data; on separate
    queues each sem lands ~0.5us after its own data.
  - a throwaway warmup indirect DMA (constant offset) runs in the idle
    ~2us window while the aux DMA is in flight, absorbing the Q7
    INDIRECT1D cold-start so both real desc-gens run warm.
  - an explicit gpsimd drain after the gathers keeps the SWDGE queues
    clean (skipping it can kill the NEXT NEFF on the core).
  - per-group 5-op DVE chain smoothl1(d)*win = mw*|d| - 0.5*mw^2 with
    mw = min(|d|, win); group-0's chain + partition-reduce matmul run
    while gather #2 is still in flight.
  - partition axis reduced on the idle PE (ones^T @ le), free-axis reduce
    out of PSUM, single 4B-descriptor output DMA whose completion is
    covered by the runtime teardown drains (no explicit wait).

Host side (everything derivable from the 14KB targets tensor): flat
gather offsets, the last-writer-wins collision mask (win) matching jax
scatter semantics, and the exact num_objects.  Descriptor slots are
padded to 8 elements (32B beats); preds_flat gets 8 zeros of padding so
the pad element of the last cell stays in bounds; pad columns are killed
by win=0.

Sharding layout per core (4 images, 2 groups of 128 targets):
  partition p in [0,128), group g in {0,1}:
    image j = g*2 + p//64 (local), target t = p%64, channel c in [0,7)
  flat offset = (gy*W + gx)*C + j*C*H*W, gy/gx = floor(coord * 5.0).

Measured on HW: 16.5us exec (baseline 21.9us), rel err 0.0e0.
"""

import numpy as np

B, C, H, W = 32, 7, 400, 400
T = 64
NCORES = 8
BLOC = B // NCORES          # 4 images per core
HW = H * W                  # 160000
CHW = C * HW                # 1120000
NELEM = BLOC * CHW          # 4480000 elements per core
PAD = 8                     # zero padding so 8-elem descriptors stay in bounds
NG = BLOC * T // 128        # 2 groups of 128 targets
P = 128
SLOT = 8                    # descriptor width in elements (32B aligned)
GC = NG * SLOT              # 16 value columns

_cached = {}
TRACE = False


def _build_nc():
    from contextlib import ExitStack

    import concourse.bacc as bacc
    import concourse.bass as bass
    import concourse.mybir as mybir

    f32 = mybir.dt.float32
    bf16 = mybir.dt.bfloat16
    i32 = mybir.dt.int32
    OP = mybir.AluOpType
    AX = mybir.AxisListType

    nc = bacc.Bacc(
        "TRN2",
        target_bir_lowering=False,
        debug=False,
        enable_asserts=False,
        num_devices=NCORES,
        num_swdge_queues=2,
    )

    def indirect_on_queue(out, in_, offset_col, queue_name):
        """indirect_dma_start with a selectable SWDGE queue.  Two indirects
        on one queue deliver BOTH completion sems only ~2us after the last
        data (they ride the queue quiesce); separate queues quiesce
        independently."""
        eng = nc.gpsimd
        out_ap = eng.lower_ap_dma(out, for_indirect_dma=True)
        in_ap = eng.lower_ap_dma(in_, for_indirect_dma=True)
        assert len(in_ap) == 1 and len(out_ap) == 1
        off_l = eng.lower_ap_dma(offset_col)
        assert len(off_l) == 1
        in_ap.append(off_l[0])
        ap_shape = in_.shape
        coef = 1
        for i in range(1, len(ap_shape)):
            coef *= ap_shape[i]
        in_ap[0].dynamic_ap_info = mybir.DynamicAccessPatternInfo(
            c=0,
            actual_ap=out.ap,
            indirect_dim_max_index=ap_shape[0],
            offset_expr=[
                mybir.DynamicAccessPatternOffsetExpr(
                    coef=coef,
                    aff_expr=mybir.DynamicAccessPatternOffsetExprAffExpr(
                        kind="IndirectArgId", arg_id=1
                    ),
                )
            ],
        )
        return eng.add_instruction(
            mybir.InstDMACopy(
                name=nc.get_next_instruction_name(),
                queue=queue_name,
                mode="Copy",
                ins=in_ap,
                outs=out_ap,
                oob_is_err=True,
                cce_op=OP.bypass,
            )
        )

    preds_flat = nc.dram_tensor(
        "preds_flat", [NELEM + PAD, 1], f32, kind="ExternalInput"
    )
    aux_i = nc.dram_tensor("aux_i", [P, NG], i32, kind="ExternalInput")
    aux_f = nc.dram_tensor("aux_f", [P, 2 * GC], f32, kind="ExternalInput")
    out_d = nc.dram_tensor("out", [1, 1], f32, kind="ExternalOutput")

    with ExitStack() as ctx:
        ec = ctx.enter_context
        oi = ec(nc.sbuf_tensor([P, NG], i32))
        xf = ec(nc.sbuf_tensor([P, 2 * GC], f32))
        gat = ec(nc.sbuf_tensor([P, GC], f32))
        d = ec(nc.sbuf_tensor([P, GC], f32))
        ad = ec(nc.sbuf_tensor([P, GC], f32))
        mw = ec(nc.sbuf_tensor([P, GC], f32))
        su = ec(nc.sbuf_tensor([P, GC], f32))
        le = ec(nc.sbuf_tensor([P, GC], bf16))
        ones = ec(nc.sbuf_tensor([P, 1], bf16))
        red = ec(nc.sbuf_tensor([1, 1], f32))
        ps = ec(nc.psum_tensor([1, GC], f32))

        dum_off = ec(nc.sbuf_tensor([P, 1], i32))
        dum_out = ec(nc.sbuf_tensor([P, SLOT], f32))

        s_ai = ec(nc.semaphore())
        s_af = ec(nc.semaphore())
        s_gat = ec(nc.semaphore())
        s_g1 = ec(nc.semaphore())
        s_v = ec(nc.semaphore())   # DVE chain counting sem (Tile-style)
        s_mm = ec(nc.semaphore())
        s_out = ec(nc.semaphore())
        s_dw = ec(nc.semaphore())
        s_dum = ec(nc.semaphore())

        # No nc.Block(): all five engine streams are emitted directly into
        # the main bb.  The Block-exit all-engine barrier is redundant with
        # the runtime's own end-of-kernel handshake (which drains each
        # engine), so skipping it shortens the tail.  The explicit gpsimd
        # drain below is the one protective piece we keep: without it the
        # SWDGE queues stay dirty and the NEXT NEFF on the core can die.
        nc.sync.dma_start(oi[:, :], aux_i[:, :]).then_inc(s_ai, 16)
        nc.sync.dma_start(xf[:, :], aux_f[:, :]).then_inc(s_af, 16)

        # gpsimd: two gathers on separate SWDGE queues.  HW indirect DMA
        # emits exactly one descriptor per partition (extra offset columns
        # are ignored), so each group is its own call; separate queues make
        # each completion sem deliver ~0.5us after its own data instead of
        # both riding the last gather's quiesce.
        # warmup: a throwaway indirect DMA absorbs the Q7 INDIRECT1D
        # cold-start inside the ~2us window while the aux DMA is in flight
        g = nc.gpsimd
        g.wait_ge(s_dw, 1)
        g.indirect_dma_start(
            out=dum_out[:, :],
            out_offset=None,
            in_=preds_flat[:, :],
            in_offset=bass.IndirectOffsetOnAxis(ap=dum_off[:, :], axis=0),
        ).then_inc(s_dum, 16)
        g.wait_ge(s_ai, 16)
        g.indirect_dma_start(
            out=gat[:, 0:SLOT],
            out_offset=None,
            in_=preds_flat[:, :],
            in_offset=bass.IndirectOffsetOnAxis(ap=oi[:, 0:1], axis=0),
        ).then_inc(s_gat, 16)
        indirect_on_queue(
            gat[:, SLOT:GC], preds_flat[:, :], oi[:, 1:2], "qPoolDynamic1"
        ).then_inc(s_g1, 16)
        g.drain()

        # vector: group-0 chain runs while gather #2 is in flight
        v = nc.vector
        v.memset(dum_off[:, :], 0).then_inc(s_dw, 1)
        v.memset(ones[:, :], 1.0).then_inc(s_v, 1)          # s_v=1
        v.wait_ge(s_af, 16)
        v.wait_ge(s_gat, 16)
        for lo, hi, sg, base in (
            (0, SLOT, None, 1),
            (SLOT, GC, s_g1, 6),
        ):
            if sg is not None:
                v.wait_ge(sg, 16)
            v.tensor_sub(
                d[:, lo:hi], gat[:, lo:hi], xf[:, lo:hi]
            ).then_inc(s_v, 1)
            v.wait_ge(s_v, base + 1)
            v.scalar_tensor_tensor(
                ad[:, lo:hi], d[:, lo:hi], -1.0, d[:, lo:hi],
                OP.mult, OP.max,
            ).then_inc(s_v, 1)
            v.wait_ge(s_v, base + 2)
            v.tensor_tensor(
                mw[:, lo:hi], ad[:, lo:hi], xf[:, GC + lo : GC + hi],
                OP.min,
            ).then_inc(s_v, 1)
            v.wait_ge(s_v, base + 3)
            v.scalar_tensor_tensor(
                su[:, lo:hi], mw[:, lo:hi], -0.5, ad[:, lo:hi],
                OP.mult, OP.add,
            ).then_inc(s_v, 1)
            v.wait_ge(s_v, base + 4)
            v.tensor_mul(
                le[:, lo:hi], mw[:, lo:hi], su[:, lo:hi]
            ).then_inc(s_v, 1)
        v.wait_ge(s_mm, 2)
        v.reduce_sum(red[:, :], ps[:, :], axis=AX.X).then_inc(s_v, 1)

        # tensor: per-group partition-reduce matmuls into disjoint PSUM cols
        t = nc.tensor
        t.wait_ge(s_v, 6)
        nc.tensor.matmul(
            ps[:, 0:SLOT], ones[:, :], le[:, 0:SLOT], start=True, stop=True
        ).then_inc(s_mm, 1)
        t.wait_ge(s_v, 11)
        nc.tensor.matmul(
            ps[:, SLOT:GC], ones[:, :], le[:, SLOT:GC], start=True, stop=True
        ).then_inc(s_mm, 1)

        # sync: single-descriptor output; completion is covered by the
        # runtime teardown drains, no explicit wait needed
        nc.sync.wait_ge(s_v, 12)
        nc.sync.dma_start(out_d[:, :], red[:, :]).then_inc(s_out, 16)

    nc.compile()
    return nc


def _get_nc():
    if "nc" not in _cached:
        _cached["nc"] = _build_nc()
    return _cached["nc"]


def _host_prep(targets):
    """Grid cells, last-writer-wins mask, and num_objects from targets only."""
    gx = np.clip(np.floor(targets[:, :, 0] * np.float32(5.0)), 0, W - 1).astype(
        np.int64
    )
    gy = np.clip(np.floor(targets[:, :, 1] * np.float32(5.0)), 0, H - 1).astype(
        np.int64
    )
    cell = gy * W + gx  # [B,T]
    win = np.zeros((B, T), np.float32)
    for b in range(B):
        last = {}
        for t in range(T):
            last[cell[b, t]] = t
        for t in last.values():
            win[b, t] = 1.0
    num = float(win.sum())
    return cell, win, num


def _regroup(x):
    """[4,T] per-image array -> [128,2] (partition p, group g) layout."""
    return np.ascontiguousarray(
        x.reshape(NG, 2, T).transpose(1, 2, 0).reshape(P, NG)
    )


def _make_in_maps(preds, targets):
    cell, win, num = _host_prep(targets)
    preds_t = np.ascontiguousarray(preds.transpose(0, 2, 3, 1))
    jbase = (np.arange(BLOC, dtype=np.int64) * CHW)[:, None]  # [4,1]
    zpad = np.zeros((PAD, 1), np.float32)

    in_maps = []
    for k in range(NCORES):
        pshard = np.concatenate(
            [preds_t[k * BLOC : (k + 1) * BLOC].reshape(NELEM, 1), zpad]
        )
        tshard = targets[k * BLOC : (k + 1) * BLOC]  # [4, 64, 7]
        tpad = np.zeros((BLOC, T, SLOT), np.float32)
        tpad[:, :, :C] = tshard
        tvp = tpad.reshape(NG, 2, T, SLOT).transpose(1, 2, 0, 3).reshape(P, GC)
        wpg = _regroup(win[k * BLOC : (k + 1) * BLOC])  # [128,2]
        winxp = np.zeros((P, GC), np.float32)
        winxp[:, 0:C] = wpg[:, 0:1]
        winxp[:, SLOT : SLOT + C] = wpg[:, 1:2]
        offs = _regroup(cell[k * BLOC : (k + 1) * BLOC] * C + jbase).astype(np.int32)
        aux_f = np.ascontiguousarray(np.hstack([tvp, winxp]).astype(np.float32))
        in_maps.append({"preds_flat": pshard, "aux_i": offs, "aux_f": aux_f})
    return in_maps, num


def kernel(preds, targets):
    from concourse.bass_utils import run_bass_kernel_spmd

    preds = np.ascontiguousarray(np.asarray(preds), dtype=np.float32)
    targets = np.ascontiguousarray(np.asarray(targets), dtype=np.float32)
    assert preds.shape == (B, C, H, W) and targets.shape == (B, T, C)

    nc = _get_nc()
    in_maps, num = _make_in_maps(preds, targets)
    res = run_bass_kernel_spmd(nc, in_maps, list(range(NCORES)), trace=TRACE)
    _cached["last_results"] = res

    lsum = np.float32(0.0)
    for k in range(NCORES):
        lsum = np.float32(lsum + np.float32(res.results[k]["out"].reshape(1)[0]))
    loss = np.float32(lsum / np.float32(np.float32(num) + np.float32(1e-6)))
    return loss, np.float32(num)


# revision 34
# speedup vs baseline: 1.2129x; 1.2129x over previous
"""Detection-loss kernel for Trainium2 (8 NeuronCores, data-parallel over B).

Reference: scatter 64 targets/image into a [B,C,H,W] map + mask, then
masked SmoothL1(preds, map).sum() / num_objects.  The mask is nonzero at
<= B*T cells, so the loss only depends on preds at those cells: each core
*gathers* preds at its 4 images' (gy,gx) cells instead of streaming 18MB.

Device program (raw Bass, hand-placed semaphores, no TileContext, no
nc.Block — engine streams are emitted straight into the main bb so the
redundant block-exit all-engine barrier disappears; the runtime's own
end-of-kernel handshake synchronizes engines):
  - two indirect DMAs (128 descriptors each, 32B slots, channels-last
    relayout so one descriptor moves all 7 channels).  HW indirect DMA
    emits exactly ONE descriptor per partition (extra offset columns are
    silently ignored - CoreSim models them, HW does not), so 256 targets
    need two calls.  They run on SEPARATE SWDGE queues (num_swdge_queues=2,
    second call hand-lowered onto qPoolDynamic1): on a shared queue both
    completion sems deliver only ~2us after the LAST data; on separate
    queues each sem lands ~0.5us after its own data.
  - a throwaway warmup indirect DMA (constant offset) runs in the idle
    ~2us window while the aux DMA is in flight, absorbing the Q7
    INDIRECT1D cold-start so both real desc-gens run warm.
  - an explicit gpsimd drain after the gathers keeps the SWDGE queues
    clean (skipping it can kill the NEXT NEFF on the core).
  - per-group 5-op DVE chain smoothl1(d)*win = mw*|d| - 0.5*mw^2 with
    mw = min(|d|, win); group-0's chain + partition-reduce matmul run
    while gather #2 is still in flight.
  - partition axis reduced on the idle PE (ones^T @ le), free-axis reduce
    out of PSUM, single 4B-descriptor output DMA whose completion is
    covered by the runtime teardown drains (no explicit wait).

Host side (everything derivable from the 14KB targets tensor): flat
gather offsets, the last-writer-wins collision mask (win) matching jax
scatter semantics, and the exact num_objects.  Descriptor slots are
padded to 8 elements (32B beats); preds_flat gets 8 zeros of padding so
the pad element of the last cell stays in bounds; pad columns are killed
by win=0.

Sharding layout per core (4 images, 2 groups of 128 targets):
  partition p in [0,128), group g in {0,1}:
    image j = g*2 + p//64 (local), target t = p%64, channel c in [0,7)
  flat offset = (gy*W + gx)*C + j*C*H*W, gy/gx = floor(coord * 5.0).

Measured on HW: 16.5us exec (baseline 21.9us), rel err 0.0e0.
"""

import numpy as np

B, C, H, W = 32, 7, 400, 400
T = 64
NCORES = 8
BLOC = B // NCORES          # 4 images per core
HW = H * W                  # 160000
CHW = C * HW                # 1120000
NELEM = BLOC * CHW          # 4480000 elements per core
PAD = 8                     # zero padding so 8-elem descriptors stay in bounds
NG = BLOC * T // 128        # 2 groups of 128 targets
P = 128
SLOT = 8                    # descriptor width in elements (32B aligned)
GC = NG * SLOT              # 16 value columns

_cached = {}
TRACE = False


def _build_nc():
    from contextlib import ExitStack

    import concourse.bacc as bacc
    import concourse.bass as bass
    import concourse.mybir as mybir

    f32 = mybir.dt.float32
    i32 = mybir.dt.int32
    OP = mybir.AluOpType
    AX = mybir.AxisListType

    nc = bacc.Bacc(
        "TRN2",
        target_bir_lowering=False,
        debug=False,
        enable_asserts=False,
        num_devices=NCORES,
        num_swdge_queues=2,
    )

    def indirect_on_queue(out, in_, offset_col, queue_name):
        """indirect_dma_start with a selectable SWDGE queue.  Two indirects
        on one queue deliver BOTH completion sems only ~2us after the last
        data (they ride the queue quiesce); separate queues quiesce
        independently."""
        eng = nc.gpsimd
        out_ap = eng.lower_ap_dma(out, for_indirect_dma=True)
        in_ap = eng.lower_ap_dma(in_, for_indirect_dma=True)
        assert len(in_ap) == 1 and len(out_ap) == 1
        off_l = eng.lower_ap_dma(offset_col)
        assert len(off_l) == 1
        in_ap.append(off_l[0])
        ap_shape = in_.shape
        coef = 1
        for i in range(1, len(ap_shape)):
            coef *= ap_shape[i]
        in_ap[0].dynamic_ap_info = mybir.DynamicAccessPatternInfo(
            c=0,
            actual_ap=out.ap,
            indirect_dim_max_index=ap_shape[0],
            offset_expr=[
                mybir.DynamicAccessPatternOffsetExpr(
                    coef=coef,
                    aff_expr=mybir.DynamicAccessPatternOffsetExprAffExpr(
                        kind="IndirectArgId", arg_id=1
                    ),
                )
            ],
        )
        return eng.add_instruction(
            mybir.InstDMACopy(
                name=nc.get_next_instruction_name(),
                queue=queue_name,
                mode="Copy",
                ins=in_ap,
                outs=out_ap,
                oob_is_err=True,
                cce_op=OP.bypass,
            )
        )

    preds_flat = nc.dram_tensor(
        "preds_flat", [NELEM + PAD, 1], f32, kind="ExternalInput"
    )
    aux_i = nc.dram_tensor("aux_i", [P, NG], i32, kind="ExternalInput")
    aux_f = nc.dram_tensor("aux_f", [P, 2 * GC], f32, kind="ExternalInput")
    out_d = nc.dram_tensor("out", [1, 1], f32, kind="ExternalOutput")

    with ExitStack() as ctx:
        ec = ctx.enter_context
        oi = ec(nc.sbuf_tensor([P, NG], i32))
        xf = ec(nc.sbuf_tensor([P, 2 * GC], f32))
        gat = ec(nc.sbuf_tensor([P, GC], f32))
        d = ec(nc.sbuf_tensor([P, GC], f32))
        ad = ec(nc.sbuf_tensor([P, GC], f32))
        mw = ec(nc.sbuf_tensor([P, GC], f32))
        su = ec(nc.sbuf_tensor([P, GC], f32))
        le = ec(nc.sbuf_tensor([P, GC], f32))
        ones = ec(nc.sbuf_tensor([P, 1], f32))
        red = ec(nc.sbuf_tensor([1, 1], f32))
        ps = ec(nc.psum_tensor([1, GC], f32))

        dum_off = ec(nc.sbuf_tensor([P, 1], i32))
        dum_out = ec(nc.sbuf_tensor([P, SLOT], f32))

        s_ai = ec(nc.semaphore())
        s_af = ec(nc.semaphore())
        s_gat = ec(nc.semaphore())
        s_g1 = ec(nc.semaphore())
        s_v = ec(nc.semaphore())   # DVE chain counting sem (Tile-style)
        s_mm = ec(nc.semaphore())
        s_out = ec(nc.semaphore())
        s_dw = ec(nc.semaphore())
        s_dum = ec(nc.semaphore())

        # No nc.Block(): all five engine streams are emitted directly into
        # the main bb.  The Block-exit all-engine barrier is redundant with
        # the runtime's own end-of-kernel handshake (which drains each
        # engine), so skipping it shortens the tail.  The explicit gpsimd
        # drain below is the one protective piece we keep: without it the
        # SWDGE queues stay dirty and the NEXT NEFF on the core can die.
        nc.sync.dma_start(oi[:, :], aux_i[:, :]).then_inc(s_ai, 16)
        nc.sync.dma_start(xf[:, :], aux_f[:, :]).then_inc(s_af, 16)

        # gpsimd: two gathers on separate SWDGE queues.  HW indirect DMA
        # emits exactly one descriptor per partition (extra offset columns
        # are ignored), so each group is its own call; separate queues make
        # each completion sem deliver ~0.5us after its own data instead of
        # both riding the last gather's quiesce.
        # warmup: a throwaway indirect DMA absorbs the Q7 INDIRECT1D
        # cold-start inside the ~2us window while the aux DMA is in flight
        g = nc.gpsimd
        g.wait_ge(s_dw, 1)
        g.indirect_dma_start(
            out=dum_out[:, :],
            out_offset=None,
            in_=preds_flat[:, :],
            in_offset=bass.IndirectOffsetOnAxis(ap=dum_off[:, :], axis=0),
        ).then_inc(s_dum, 16)
        g.wait_ge(s_ai, 16)
        g.indirect_dma_start(
            out=gat[:, 0:SLOT],
            out_offset=None,
            in_=preds_flat[:, :],
            in_offset=bass.IndirectOffsetOnAxis(ap=oi[:, 0:1], axis=0),
        ).then_inc(s_gat, 16)
        indirect_on_queue(
            gat[:, SLOT:GC], preds_flat[:, :], oi[:, 1:2], "qPoolDynamic1"
        ).then_inc(s_g1, 16)
        g.drain()

        # vector: group-0 chain runs while gather #2 is in flight
        v = nc.vector
        v.memset(dum_off[:, :], 0).then_inc(s_dw, 1)
        v.memset(ones[:, :], 1.0).then_inc(s_v, 1)          # s_v=1
        v.wait_ge(s_af, 16)
        v.wait_ge(s_gat, 16)
        for lo, hi, sg, base in (
            (0, SLOT, None, 1),
            (SLOT, GC, s_g1, 6),
        ):
            if sg is not None:
                v.wait_ge(sg, 16)
            v.tensor_sub(
                d[:, lo:hi], gat[:, lo:hi], xf[:, lo:hi]
            ).then_inc(s_v, 1)
            v.wait_ge(s_v, base + 1)
            v.scalar_tensor_tensor(
                ad[:, lo:hi], d[:, lo:hi], -1.0, d[:, lo:hi],
                OP.mult, OP.max,
            ).then_inc(s_v, 1)
            v.wait_ge(s_v, base + 2)
            v.tensor_tensor(
                mw[:, lo:hi], ad[:, lo:hi], xf[:, GC + lo : GC + hi],
                OP.min,
            ).then_inc(s_v, 1)
            v.wait_ge(s_v, base + 3)
            v.scalar_tensor_tensor(
                su[:, lo:hi], mw[:, lo:hi], -0.5, ad[:, lo:hi],
                OP.mult, OP.add,
            ).then_inc(s_v, 1)
            v.wait_ge(s_v, base + 4)
            v.tensor_mul(
                le[:, lo:hi], mw[:, lo:hi], su[:, lo:hi]
            ).then_inc(s_v, 1)
        v.wait_ge(s_mm, 2)
        v.reduce_sum(red[:, :], ps[:, :], axis=AX.X).then_inc(s_v, 1)

        # tensor: per-group partition-reduce matmuls into disjoint PSUM cols
        t = nc.tensor
        t.wait_ge(s_v, 6)
        nc.tensor.matmul(
            ps[:, 0:SLOT], ones[:, :], le[:, 0:SLOT], start=True, stop=True
        ).then_inc(s_mm, 1)
        t.wait_ge(s_v, 11)
        nc.tensor.matmul(
            ps[:, SLOT:GC], ones[:, :], le[:, SLOT:GC], start=True, stop=True
        ).then_inc(s_mm, 1)

        # sync: single-descriptor output; completion is covered by the
        # runtime teardown drains, no explicit wait needed
        nc.sync.wait_ge(s_v, 12)
        nc.sync.dma_start(out_d[:, :], red[:, :]).then_inc(s_out, 16)

    nc.compile()
    return nc


def _get_nc():
    if "nc" not in _cached:
        _cached["nc"] = _build_nc()
    return _cached["nc"]


def _host_prep(targets):
    """Grid cells, last-writer-wins mask, and num_objects from targets only."""
    gx = np.clip(np.floor(targets[:, :, 0] * np.float32(5.0)), 0, W - 1).astype(
        np.int64
    )
    gy = np.clip(np.floor(targets[:, :, 1] * np.float32(5.0)), 0, H - 1).astype(
        np.int64
    )
    cell = gy * W + gx  # [B,T]
    win = np.zeros((B, T), np.float32)
    for b in range(B):
        last = {}
        for t in range(T):
            last[cell[b, t]] = t
        for t in last.values():
            win[b, t] = 1.0
    num = float(win.sum())
    return cell, win, num


def _regroup(x):
    """[4,T] per-image array -> [128,2] (partition p, group g) layout."""
    return np.ascontiguousarray(
        x.reshape(NG, 2, T).transpose(1, 2, 0).reshape(P, NG)
    )


def _make_in_maps(preds, targets):
    cell, win, num = _host_prep(targets)
    preds_t = np.ascontiguousarray(preds.transpose(0, 2, 3, 1))
    jbase = (np.arange(BLOC, dtype=np.int64) * CHW)[:, None]  # [4,1]
    zpad = np.zeros((PAD, 1), np.float32)

    in_maps = []
    for k in range(NCORES):
        pshard = np.concatenate(
            [preds_t[k * BLOC : (k + 1) * BLOC].reshape(NELEM, 1), zpad]
        )
        tshard = targets[k * BLOC : (k + 1) * BLOC]  # [4, 64, 7]
        tpad = np.zeros((BLOC, T, SLOT), np.float32)
        tpad[:, :, :C] = tshard
        tvp = tpad.reshape(NG, 2, T, SLOT).transpose(1, 2, 0, 3).reshape(P, GC)
        wpg = _regroup(win[k * BLOC : (k + 1) * BLOC])  # [128,2]
        winxp = np.zeros((P, GC), np.float32)
        winxp[:, 0:C] = wpg[:, 0:1]
        winxp[:, SLOT : SLOT + C] = wpg[:, 1:2]
        offs = _regroup(cell[k * BLOC : (k + 1) * BLOC] * C + jbase).astype(np.int32)
        aux_f = np.ascontiguousarray(np.hstack([tvp, winxp]).astype(np.float32))
        in_maps.append({"preds_flat": pshard, "aux_i": offs, "aux_f": aux_f})
    return in_maps, num


def kernel(preds, targets):
    from concourse.bass_utils import run_bass_kernel_spmd

    preds = np.ascontiguousarray(np.asarray(preds), dtype=np.float32)
    targets = np.ascontiguousarray(np.asarray(targets), dtype=np.float32)
    assert preds.shape == (B, C, H, W) and targets.shape == (B, T, C)

    nc = _get_nc()
    in_maps, num = _make_in_maps(preds, targets)
    res = run_bass_kernel_spmd(nc, in_maps, list(range(NCORES)), trace=TRACE)
    _cached["last_results"] = res

    lsum = np.float32(0.0)
    for k in range(NCORES):
        lsum = np.float32(lsum + np.float32(res.results[k]["out"].reshape(1)[0]))
    loss = np.float32(lsum / np.float32(np.float32(num) + np.float32(1e-6)))
    return loss, np.float32(num)
